# revision 1
# baseline (speedup 1.0000x reference)
"""Local (windowed) attention with shared KV head — TRN2 Bass kernel.

Problem: b=1, L=4096, d_model=1024, n_head=16, d_head=64, w=512.
  qp = (q@Wq)/8; k,v = kv@Wkv; per 512-chunk attention over {prev,self,next}
  chunks with zero-padded edges (softmax includes exp(0)=1 terms for pads);
  out = ctx @ Wo.

Sharding: sequence-parallel over the 8 chunks, one chunk per NeuronCore.
Each core recomputes the K/V projection for its 3-chunk halo (no
collectives). Edge cores receive zero-filled halo slices, which reproduces
the reference's zero-padding exactly (scores 0 -> exp 1 in the softmax).

Per-core dataflow (all matmuls in float32r = full-rate PE, ~1e-4 rel err):
  kvp^T = [Wv|Wk]^T @ kv^T            (24 MMs)   -> vT (rows 0:64), kT (64:128)
  k3T2  = kT duplicated to both partition halves (SBUF->SBUF DMA)
  v65   = PE-transpose(vT) with a ones column appended   ([y,64+1] tiles)
  qp^T  = (Wq/8)^T @ q^T              (64 MMs)   -> 8 tiles [128,512], head pair per tile
  scores: S^T[y,x] per head, row-packed pairs (2 heads share the PE array)
  P^T   = exp(S^T) on ScalarE, PSUM->SBUF, [128,1024] groups
  ctx^T+Z = [v|1]^T @ P^T fused       (M=65: rows 0:64 ctx, row 64 = softmax denom)
  norm  : zinv=1/Z; broadcast via K=1 matmul; ctxn = ctx * zinv_bcast
  out   = ctxn^T-tiles (lhsT) @ Wo    (64 MMs)   -> [512,1024] row-major -> DMA
"""

import numpy as np

B, L, DM, NH, DH, W = 1, 4096, 1024, 16, 64, 512
NCORES = 8
CH = L // NCORES        # 512 tokens per core
YW = 3 * W              # 1536 halo positions
P = 128
NF = DM // P            # 8 feature tiles
NY = YW // P            # 12 y tiles
NPAIR = NH // 2         # 8 head pairs
NGRP = NY // 2          # 6 score groups of 2 y-tiles

_CACHE = {}


def _build():
    import concourse.mybir as mybir
    import concourse.tile as tile
    from concourse import bacc
    from concourse.masks import make_identity
    from contextlib import ExitStack

    F32 = mybir.dt.float32
    F32R = mybir.dt.float32r
    EXP = mybir.ActivationFunctionType.Exp

    nc = bacc.Bacc("TRN2", target_bir_lowering=False, debug=False)
    QT = nc.dram_tensor("QT", [DM, CH], F32R, kind="ExternalInput")
    KVT = nc.dram_tensor("KVT", [DM, YW], F32R, kind="ExternalInput")
    WQ = nc.dram_tensor("WQ", [DM, DM], F32R, kind="ExternalInput")     # pre-scaled by 1/8
    WVK = nc.dram_tensor("WVK", [DM, P], F32R, kind="ExternalInput")    # [Wv | Wk]
    WO = nc.dram_tensor("WO", [DM, DM], F32R, kind="ExternalInput")
    OUT = nc.dram_tensor("OUT", [CH, DM], F32, kind="ExternalOutput")

    with tile.TileContext(nc) as tc, ExitStack() as ctx:
        perm = ctx.enter_context(tc.tile_pool(name="perm", bufs=1))

        identf = perm.tile([P, P], F32, tag="identf")
        make_identity(nc, identf[:])
        onesb = perm.tile([P, P], F32R, tag="onesb")
        nc.vector.memset(onesb[:].bitcast(F32), 1.0)

        # --- persistent SBUF tiles
        wvk = [perm.tile([P, P], F32R, tag=f"wvk{f}", name=f"wvk{f}") for f in range(NF)]
        wq = [perm.tile([P, DM], F32R, tag=f"wq{f}", name=f"wq{f}") for f in range(NF)]
        wo = [perm.tile([P, DM], F32R, tag=f"wo{f}", name=f"wo{f}") for f in range(NF)]
        k3T2 = perm.tile([P, YW], F32R, tag="k3T2")
        vTs = perm.tile([64, YW], F32, tag="vTs")
        v65 = [perm.tile([P, 65], F32R, tag=f"v65_{t}", name=f"v65_{t}") for t in range(NY)]
        qpT = [perm.tile([P, CH], F32R, tag=f"qpT{m}", name=f"qpT{m}") for m in range(NF)]
        ctxn = [perm.tile([P, CH], F32R, tag=f"ctxn{i}", name=f"ctxn{i}") for i in range(NPAIR)]

        for f in range(NF):
            nc.sync.dma_start(wvk[f][:], WVK.ap()[P * f:P * (f + 1), :])

        with tc.tile_pool(name="kvt", bufs=1) as kvtp, \
             tc.tile_pool(name="ph0ps", bufs=3, space="PSUM") as ph0, \
             tc.tile_pool(name="tpps", bufs=2, space="PSUM") as tpp:
            kvt = [kvtp.tile([P, YW], F32R, tag=f"kvt{f}", name=f"kvt{f}") for f in range(NF)]
            for f in range(NF):
                nc.sync.dma_start(kvt[f][:], KVT.ap()[P * f:P * (f + 1), :])
            # kv projection: [128,512] psum per n-tile; rows 0:64=vT, 64:128=kT
            for n in range(3):
                ps = ph0.tile([P, W], F32, tag="kvp")
                for f in range(NF):
                    nc.tensor.matmul(ps[:], wvk[f][:], kvt[f][:, W * n:W * (n + 1)],
                                     start=(f == 0), stop=(f == NF - 1))
                ns = slice(W * n, W * (n + 1))
                nc.vector.tensor_copy(vTs[:, ns], ps[0:64, :])
                nc.vector.tensor_copy(k3T2[64:128, ns], ps[64:128, :])
            # duplicate kT into the low partition half (partition remap DMA)
            nc.sync.dma_start(k3T2[0:64, :], k3T2[64:128, :])
            # v65 tiles: PE transpose of vT + ones column
            for t in range(NY):
                tp = tpp.tile([P, 64], F32, tag="tp")
                nc.tensor.transpose(tp[:], vTs[:, P * t:P * (t + 1)],
                                    identf[0:64, 0:64])
                nc.vector.tensor_copy(v65[t][:, 0:64], tp[:])
                nc.vector.memset(v65[t][:, 64:65].bitcast(F32), 1.0)

        # --- q projection
        with tc.tile_pool(name="qt", bufs=1) as qtp, \
             tc.tile_pool(name="qpps", bufs=8, space="PSUM") as qpp:
            qt = [qtp.tile([P, CH], F32R, tag=f"qt{f}", name=f"qt{f}") for f in range(NF)]
            for f in range(NF):
                nc.sync.dma_start(qt[f][:], QT.ap()[P * f:P * (f + 1), :])
            for f in range(NF):
                nc.sync.dma_start(wq[f][:], WQ.ap()[P * f:P * (f + 1), :])
            for m in range(NF):
                ps = qpp.tile([P, CH], F32, tag="qp")
                for f in range(NF):
                    nc.tensor.matmul(ps[:], wq[f][:, P * m:P * (m + 1)], qt[f][:],
                                     start=(f == 0), stop=(f == NF - 1))
                nc.vector.tensor_copy(qpT[m][:], ps[:])

        for f in range(NF):
            nc.sync.dma_start(wo[f][:], WO.ap()[P * f:P * (f + 1), :])

        # --- attention per head pair
        with tc.tile_pool(name="scps", bufs=2, space="PSUM") as scp, \
             tc.tile_pool(name="cxps", bufs=3, space="PSUM") as cxp, \
             tc.tile_pool(name="pt", bufs=4) as ptp, \
             tc.tile_pool(name="zn", bufs=4) as znp:
            for i in range(NPAIR):
                cxA = cxp.tile([P, W], F32, tag="cx")
                cxB = cxp.tile([P, W], F32, tag="cx")
                for g in range(NGRP):
                    scA = scp.tile([P, 2 * W], F32, tag="sc")
                    scB = scp.tile([P, 2 * W], F32, tag="sc")
                    for t in range(2):
                        y = 2 * g + t
                        ys = slice(P * y, P * (y + 1))
                        ts_ = slice(W * t, W * (t + 1))
                        nc.tensor.matmul(scA[:, ts_], k3T2[0:64, ys],
                                         qpT[i][0:64, :], start=True, stop=True,
                                         tile_position=(0, 0))
                        nc.tensor.matmul(scB[:, ts_], k3T2[64:128, ys],
                                         qpT[i][64:128, :], start=True, stop=True,
                                         tile_position=(64, 0))
                    pA = ptp.tile([P, 2 * W], F32R, tag="pt")
                    pB = ptp.tile([P, 2 * W], F32R, tag="pt")
                    nc.scalar.activation(pA[:], scA[:], EXP)
                    nc.scalar.activation(pB[:], scB[:], EXP)
                    for t in range(2):
                        y = 2 * g + t
                        ts_ = slice(W * t, W * (t + 1))
                        st = (g == 0 and t == 0)
                        sp = (g == NGRP - 1 and t == 1)
                        nc.tensor.matmul(cxA[0:65, :], v65[y][:], pA[:, ts_],
                                         start=st, stop=sp)
                        nc.tensor.matmul(cxB[0:65, :], v65[y][:], pB[:, ts_],
                                         start=st, stop=sp)
                # normalize: ctxn[i][0:64] = cxA/Z_A ; [64:128] = cxB/Z_B (via DMA)
                for h, cx in ((0, cxA), (1, cxB)):
                    zinv = znp.tile([65, W], F32R, tag="zinv")
                    with nc.allow_low_precision(reason="softmax denom feeds f32r matmul"):
                        nc.vector.reciprocal(zinv[64:65, :], cx[64:65, :])
                    zbc = cxp.tile([P, W], F32, tag="cx")
                    nc.tensor.matmul(zbc[0:64, :], onesb[64:65, 0:64],
                                     zinv[64:65, :], start=True, stop=True,
                                     tile_position=(64, 0))
                    cxs = znp.tile([64, W], F32, tag="cxs")
                    nc.vector.tensor_copy(cxs[:], cx[0:64, :])
                    if h == 0:
                        with nc.allow_low_precision(reason="ctx feeds f32r matmul"):
                            nc.vector.tensor_mul(ctxn[i][0:64, :], cxs[:],
                                                 zbc[0:64, :])
                    else:
                        cbt = znp.tile([64, W], F32R, tag="cbt")
                        with nc.allow_low_precision(reason="ctx feeds f32r matmul"):
                            nc.vector.tensor_mul(cbt[:], cxs[:], zbc[0:64, :])
                        nc.sync.dma_start(ctxn[i][64:128, :], cbt[:])

        # --- output projection: out[x,o] = sum_i ctxn[i][:,x].T @ wo[i][:,o]
        with tc.tile_pool(name="opps", bufs=8, space="PSUM") as opp, \
             tc.tile_pool(name="osb", bufs=4) as osb:
            for x in range(4):
                xs = slice(P * x, P * (x + 1))
                for o in range(2):
                    os_ = slice(W * o, W * (o + 1))
                    ps = opp.tile([P, W], F32, tag="op")
                    for i in range(NPAIR):
                        nc.tensor.matmul(ps[:], ctxn[i][:, xs], wo[i][:, os_],
                                         start=(i == 0), stop=(i == NPAIR - 1))
                    ot = osb.tile([P, W], F32, tag="os")
                    nc.scalar.copy(ot[:], ps[:])
                    nc.sync.dma_start(OUT.ap()[xs, os_], ot[:])

    nc.compile()
    return nc


def _get_nc():
    if "nc" not in _CACHE:
        _CACHE["nc"] = _build()
    return _CACHE["nc"]


def kernel(q, kv, Wq, Wkv, Wo, w=None, _trace=False):
    from concourse import bass_utils

    q = np.asarray(q, np.float32).reshape(L, DM)
    kv = np.asarray(kv, np.float32).reshape(L, DM)
    Wq = np.asarray(Wq, np.float32)
    Wkv = np.asarray(Wkv, np.float32)
    Wo = np.asarray(Wo, np.float32)

    qT = np.ascontiguousarray(q.T)                      # [DM, L]
    kvT = np.ascontiguousarray(kv.T)                    # [DM, L]
    WQs = np.ascontiguousarray(Wq / np.sqrt(DH))        # fold 1/sqrt(d_head)
    WVK = np.ascontiguousarray(
        np.concatenate([Wkv[:, DH:], Wkv[:, :DH]], axis=1))  # [Wv | Wk]

    in_maps = []
    for c in range(NCORES):
        kvt_c = np.zeros((DM, YW), np.float32)
        lo = (c - 1) * CH
        hi = (c + 2) * CH
        src_lo, src_hi = max(lo, 0), min(hi, L)
        dst_lo = src_lo - lo
        kvt_c[:, dst_lo:dst_lo + (src_hi - src_lo)] = kvT[:, src_lo:src_hi]
        in_maps.append({
            "QT": np.ascontiguousarray(qT[:, c * CH:(c + 1) * CH]),
            "KVT": kvt_c,
            "WQ": WQs,
            "WVK": WVK,
            "WO": Wo,
        })

    nc = _get_nc()
    res = bass_utils.run_bass_kernel_spmd(
        nc, in_maps, core_ids=list(range(NCORES)), trace=_trace)
    if _trace:
        _CACHE["last_result"] = res

    out = np.concatenate([r["OUT"] for r in res.results], axis=0)
    return out.reshape(B, L, DM).astype(np.float32)



# revision 10
# speedup vs baseline: 1.4454x; 1.4454x over previous
"""Local (windowed) attention with shared KV head — TRN2 Bass kernel, v2.

Problem: b=1, L=4096, d_model=1024, n_head=16, d_head=64, w=512.
  qp = (q@Wq)/8; k,v = kv@Wkv; per 512-chunk attention over {prev,self,next}
  chunks with zero-padded edges (softmax includes exp(0)=1 terms for pads);
  out = ctx @ Wo.

Sharding: sequence-parallel over the 8 chunks, one chunk per NeuronCore.
Each core recomputes the K/V projection for its 3-chunk halo (no
collectives). Edge cores receive zero-filled halo slices, which reproduces
the reference's zero-padding exactly.

v2 changes vs the fp32r baseline (322us):
  - all matmuls bf16 (fp32r streams at ~2 cyc/row on HW; bf16 at 1)
  - exp writes bf16 probs directly from ScalarE
  - softmax 1/Z via exp(-ln Z) on ScalarE (same act table set as exp),
    batched over heads; kills the 16x 3.4us DVE RECIPROCALs that stalled
    the PE and caused HAM re-throttling to 1.2 GHz for ~70% of the kernel
  - f-outer projection loops consume DMA'd tiles on arrival
  - q-projection and softmax normalization interleaved into the
    ACT-bound attention phase; output projection i-outer at the tail

Per-core dataflow:
  kvp/qp01: f-outer accumulation in 5 PSUM banks, DMA-paced
  k3T2  = kT duplicated to both partition halves (SBUF->SBUF DMA)
  v65   = PE-transpose(vT) with a ones column appended ([y,64+1] tiles)
  scores: S^T[y,x] per head, row-packed pairs co-execute on the PE
  P^T   = exp(S^T) on ScalarE -> bf16 SBUF [128,1024] tiles
  ctx^T+Z = [v|1]^T @ P^T fused (M=65: rows 0:64 ctx, row 64 = denom Z)
  norm  : Z rows gathered into [16,512]; zinv = exp(-ln Z) on ScalarE;
          broadcast via K=16 selection-matrix matmuls; ctxn = ctx * zinv
  out   = ctxn-tiles (lhsT) @ Wo, i-outer over all 8 PSUM banks
"""

import numpy as np

B, L, DM, NH, DH, W = 1, 4096, 1024, 16, 64, 512
NCORES = 8
CH = L // NCORES        # 512 tokens per core
YW = 3 * W              # 1536 halo positions
P = 128
NF = DM // P            # 8 feature tiles
NY = YW // P            # 12 y tiles
NPAIR = NH // 2         # 8 head pairs
NGRP = NY // 2          # 6 score groups of 2 y-tiles

_CACHE = {}


def _build():
    import concourse.mybir as mybir
    import concourse.tile as tile
    from concourse import bacc
    from concourse.masks import make_identity
    from contextlib import ExitStack

    F32 = mybir.dt.float32
    BF16 = mybir.dt.bfloat16
    EXP = mybir.ActivationFunctionType.Exp
    LN = mybir.ActivationFunctionType.Ln

    nc = bacc.Bacc("TRN2", target_bir_lowering=False, debug=False)
    QT = nc.dram_tensor("QT", [DM, CH], BF16, kind="ExternalInput")
    KVT = nc.dram_tensor("KVT", [DM, YW], BF16, kind="ExternalInput")
    WQ = nc.dram_tensor("WQ", [DM, DM], BF16, kind="ExternalInput")    # pre-scaled by 1/8
    WVK = nc.dram_tensor("WVK", [DM, P], BF16, kind="ExternalInput")   # [Wv | Wk]
    WO = nc.dram_tensor("WO", [DM, DM], BF16, kind="ExternalInput")
    SEL = nc.dram_tensor("SEL", [36, P * NPAIR], BF16, kind="ExternalInput")
    OUT = nc.dram_tensor("OUT", [CH, DM], F32, kind="ExternalOutput")

    with tile.TileContext(nc) as tc, ExitStack() as ctx:
        perm = ctx.enter_context(tc.tile_pool(name="perm", bufs=1))

        identb = perm.tile([64, 64], BF16, tag="identb")
        make_identity(nc, identb[:])

        # --- persistent SBUF tiles
        wvk = [perm.tile([P, P], BF16, tag=f"wvk{f}", name=f"wvk{f}") for f in range(NF)]
        wq = [perm.tile([P, DM], BF16, tag=f"wq{f}", name=f"wq{f}") for f in range(NF)]
        wo = [perm.tile([P, DM], BF16, tag=f"wo{f}", name=f"wo{f}") for f in range(NF)]
        qt = [perm.tile([P, CH], BF16, tag=f"qt{f}", name=f"qt{f}") for f in range(NF)]
        sel = perm.tile([36, P * NPAIR], BF16, tag="sel")
        k3T2 = perm.tile([P, YW], BF16, tag="k3T2")
        vTs = perm.tile([64, YW], BF16, tag="vTs")
        v65 = [perm.tile([P, 65], BF16, tag=f"v65_{t}", name=f"v65_{t}") for t in range(NY)]
        qpT = [perm.tile([P, CH], BF16, tag=f"qpT{m}", name=f"qpT{m}") for m in range(NF)]
        # pre-normalization ctx (f32) + denominator staging, per head
        stq = [perm.tile([65, W], F32, tag=f"stq{h}", name=f"stq{h}") for h in range(NH)]
        # Z rows: pairs 0..5 at partitions 0:12; pairs 6,7 at 32:36 (ScalarE
        # partition bases must be 32-aligned, so the two batches are split)
        zpack = perm.tile([36, W], F32, tag="zpack")
        zlog = perm.tile([36, W], F32, tag="zlog")
        zinv = perm.tile([36, W], BF16, tag="zinv")
        # normalized ctx pair tiles (lhsT of the output projection)
        pairctx = [perm.tile([P, W], BF16, tag=f"pctx{i}", name=f"pctx{i}")
                   for i in range(NPAIR)]

        # ---------- preamble: DMAs + kv/q projections, DMA-paced f-outer ----
        with ExitStack() as pre:
            kvtp = pre.enter_context(tc.tile_pool(name="kvt", bufs=1))
            kvps = pre.enter_context(tc.tile_pool(name="kvps", bufs=1, space="PSUM"))
            qps01 = pre.enter_context(tc.tile_pool(name="qps01", bufs=1, space="PSUM"))
            tpp = pre.enter_context(tc.tile_pool(name="tpps", bufs=2, space="PSUM"))

            kvt = [kvtp.tile([P, YW], BF16, tag=f"kvt{f}", name=f"kvt{f}")
                   for f in range(NF)]
            for f in range(NF):
                nc.sync.dma_start(wvk[f][:], WVK.ap()[P * f:P * (f + 1), :])
            for f in range(NF):
                nc.sync.dma_start(kvt[f][:], KVT.ap()[P * f:P * (f + 1), :])
                nc.sync.dma_start(qt[f][:], QT.ap()[P * f:P * (f + 1), :])
                nc.sync.dma_start(wq[f][:], WQ.ap()[P * f:P * (f + 1), :])
            nc.sync.dma_start(sel[:], SEL.ap()[:, :])
            for f in range(NF):
                nc.sync.dma_start(wo[f][:], WO.ap()[P * f:P * (f + 1), :])

            ps_n = [kvps.tile([P, W], F32, tag=f"n{n}", name=f"kvpn{n}")
                    for n in range(3)]
            qp01 = [qps01.tile([P, CH], F32, tag=f"q{m}", name=f"qp01_{m}")
                    for m in range(2)]
            for f in range(NF):
                st, sp = (f == 0), (f == NF - 1)
                for n in range(3):
                    nc.tensor.matmul(ps_n[n][:], wvk[f][:],
                                     kvt[f][:, W * n:W * (n + 1)],
                                     start=st, stop=sp, skip_group_check=True)
                for m in range(2):
                    nc.tensor.matmul(qp01[m][:], wq[f][:, P * m:P * (m + 1)],
                                     qt[f][:], start=st, stop=sp,
                                     skip_group_check=True)

            for n in range(3):
                ns = slice(W * n, W * (n + 1))
                nc.vector.tensor_copy(vTs[:, ns], ps_n[n][0:64, :])
                nc.vector.tensor_copy(k3T2[64:128, ns], ps_n[n][64:128, :])
            for m in range(2):
                nc.vector.tensor_copy(qpT[m][:], qp01[m][:])
            # duplicate kT into the low partition half (partition remap DMA)
            nc.sync.dma_start(k3T2[0:64, :], k3T2[64:128, :])
            # v65 tiles: PE transpose of vT + ones column
            for t in range(NY):
                tp = tpp.tile([P, 64], BF16, tag="tp")
                nc.tensor.transpose(tp[:], vTs[:, P * t:P * (t + 1)],
                                    identb[:])
                nc.vector.tensor_copy(v65[t][:, 0:64], tp[:])
                nc.vector.memset(v65[t][:, 64:65], 1.0)

        # ---------- attention ------------------------------------------------
        with ExitStack() as att:
            scp = att.enter_context(tc.tile_pool(name="scps", bufs=2, space="PSUM"))
            cxp = att.enter_context(tc.tile_pool(name="cxps", bufs=2, space="PSUM"))
            ptp = att.enter_context(tc.tile_pool(name="pt", bufs=4))
            tbp = att.enter_context(tc.tile_pool(name="tbp", bufs=2))
            qpx = ExitStack()
            qpp = qpx.enter_context(tc.tile_pool(name="qpps", bufs=1, space="PSUM"))
            zbcp = None

            def norm_pair(j):
                # zinv rows for heads 2j / 2j+1 broadcast to 64 partitions via
                # selection-matrix matmuls, then ctxn = ctx * zinv
                zbA = zbcp.tile([64, W], F32, tag="zb")
                zbB = zbcp.tile([64, W], F32, tag="zb")
                rows = slice(0, 12) if j < 6 else slice(32, 36)
                nc.tensor.matmul(zbA[:], sel[rows, P * j:P * j + 64],
                                 zinv[rows, :],
                                 start=True, stop=True, skip_group_check=True)
                nc.tensor.matmul(zbB[:], sel[rows, P * j + 64:P * (j + 1)],
                                 zinv[rows, :],
                                 start=True, stop=True, skip_group_check=True)
                with nc.allow_low_precision(reason="ctx feeds bf16 matmul"):
                    nc.vector.tensor_mul(pairctx[j][0:64, :], stq[2 * j][0:64, :],
                                         zbA[:])
                    tmpB = tbp.tile([64, W], BF16, tag="tb")
                    nc.vector.tensor_mul(tmpB[:], stq[2 * j + 1][0:64, :], zbB[:])
                nc.sync.dma_start(pairctx[j][64:128, :], tmpB[:])

            for i in range(NPAIR):
                cxA = cxp.tile([65, W], F32, tag="cx")
                cxB = cxp.tile([65, W], F32, tag="cx")
                for g in range(NGRP):
                    scA = scp.tile([P, 2 * W], F32, tag="sc")
                    scB = scp.tile([P, 2 * W], F32, tag="sc")
                    for t in range(2):
                        y = 2 * g + t
                        ys = slice(P * y, P * (y + 1))
                        ts_ = slice(W * t, W * (t + 1))
                        nc.tensor.matmul(scA[:, ts_], k3T2[0:64, ys],
                                         qpT[i][0:64, :], start=True, stop=True,
                                         tile_position=(0, 0),
                                         skip_group_check=True)
                        nc.tensor.matmul(scB[:, ts_], k3T2[64:128, ys],
                                         qpT[i][64:128, :], start=True, stop=True,
                                         tile_position=(64, 0),
                                         skip_group_check=True)
                    pA = ptp.tile([P, 2 * W], BF16, tag="pt")
                    pB = ptp.tile([P, 2 * W], BF16, tag="pt")
                    nc.scalar.activation(pA[:], scA[:], EXP)
                    nc.scalar.activation(pB[:], scB[:], EXP)
                    for t in range(2):
                        y = 2 * g + t
                        ts_ = slice(W * t, W * (t + 1))
                        st = (g == 0 and t == 0)
                        sp = (g == NGRP - 1 and t == 1)
                        nc.tensor.matmul(cxA[:], v65[y][:], pA[:, ts_],
                                         start=st, stop=sp,
                                         skip_group_check=True)
                        nc.tensor.matmul(cxB[:], v65[y][:], pB[:, ts_],
                                         start=st, stop=sp,
                                         skip_group_check=True)
                    # pair 7's groups carry the normalization of pairs 0..5
                    if i == NPAIR - 1:
                        norm_pair(g)
                # stage ctx+Z out of PSUM; gather Z rows into zpack
                stA, stB = stq[2 * i], stq[2 * i + 1]
                nc.vector.tensor_copy(stA[:], cxA[:])
                nc.vector.tensor_copy(stB[:], cxB[:])
                zr = 2 * i if i < 6 else 32 + 2 * (i - 6)
                nc.sync.dma_start(zpack[zr:zr + 1, :], stA[64:65, :])
                nc.sync.dma_start(zpack[zr + 1:zr + 2, :], stB[64:65, :])
                # q-projection for pair i+2, interleaved
                m = i + 2
                if 2 <= m < NF:
                    qps = qpp.tile([P, CH], F32, tag="qp")
                    for f in range(NF):
                        nc.tensor.matmul(qps[:], wq[f][:, P * m:P * (m + 1)],
                                         qt[f][:], start=(f == 0),
                                         stop=(f == NF - 1),
                                         skip_group_check=True)
                    nc.vector.tensor_copy(qpT[m][:], qps[:])
                if i == NPAIR - 3:
                    qpx.close()   # free the qp PSUM bank for zbc tiles
                if i == NPAIR - 2:
                    # zinv for pairs 0..5 (Z rows 0:12), batched on ScalarE
                    nc.scalar.activation(zlog[0:12, :], zpack[0:12, :], LN)
                    nc.scalar.activation(zinv[0:12, :], zlog[0:12, :], EXP,
                                         scale=-1.0)
                    zbcp = att.enter_context(
                        tc.tile_pool(name="zbcp", bufs=2, space="PSUM"))

            # pairs 6, 7: zinv (Z rows 32:36) + normalization
            nc.scalar.activation(zlog[32:36, :], zpack[32:36, :], LN)
            nc.scalar.activation(zinv[32:36, :], zlog[32:36, :], EXP, scale=-1.0)
            norm_pair(NPAIR - 2)
            norm_pair(NPAIR - 1)

        # ---------- output projection: out[x,o] = sum_i ctxn[i].T @ wo[i] ----
        with tc.tile_pool(name="opps", bufs=1, space="PSUM") as opp, \
             tc.tile_pool(name="osb", bufs=4) as osb:
            ops = [[opp.tile([P, W], F32, tag=f"o{x}{o}", name=f"o{x}{o}")
                    for o in range(2)] for x in range(4)]
            for i in range(NPAIR):
                for x in range(4):
                    xs = slice(P * x, P * (x + 1))
                    for o in range(2):
                        os_ = slice(W * o, W * (o + 1))
                        nc.tensor.matmul(ops[x][o][:], pairctx[i][:, xs],
                                         wo[i][:, os_], start=(i == 0),
                                         stop=(i == NPAIR - 1),
                                         skip_group_check=True)
            for x in range(4):
                xs = slice(P * x, P * (x + 1))
                for o in range(2):
                    os_ = slice(W * o, W * (o + 1))
                    ot = osb.tile([P, W], F32, tag="os")
                    nc.scalar.copy(ot[:], ops[x][o][:])
                    nc.sync.dma_start(OUT.ap()[xs, os_], ot[:])

    nc.compile()
    return nc


def _get_nc():
    if "nc" not in _CACHE:
        _CACHE["nc"] = _build()
    return _CACHE["nc"]


def kernel(q, kv, Wq, Wkv, Wo, w=None, _trace=False):
    import ml_dtypes
    from concourse import bass_utils

    BF = ml_dtypes.bfloat16

    q = np.asarray(q, np.float32).reshape(L, DM)
    kv = np.asarray(kv, np.float32).reshape(L, DM)
    Wq = np.asarray(Wq, np.float32)
    Wkv = np.asarray(Wkv, np.float32)
    Wo = np.asarray(Wo, np.float32)

    qT = np.ascontiguousarray(q.T).astype(BF)               # [DM, L]
    kvT = np.ascontiguousarray(kv.T)                        # [DM, L] f32
    WQs = np.ascontiguousarray(Wq / np.sqrt(DH)).astype(BF)
    WVK = np.ascontiguousarray(
        np.concatenate([Wkv[:, DH:], Wkv[:, :DH]], axis=1)).astype(BF)
    WOb = np.ascontiguousarray(Wo).astype(BF)

    # selection matrix: picks the zinv row of head 2j/2j+1 for pair j's
    # broadcast matmul. Pairs 0..5 use zinv rows 2j/2j+1; pairs 6,7 use
    # rows 32+2(j-6)/33+2(j-6) (32-aligned ScalarE batches).
    selm = np.zeros((36, P * NPAIR), np.float32)
    for j in range(NPAIR):
        r = 2 * j if j < 6 else 32 + 2 * (j - 6)
        selm[r, P * j:P * j + 64] = 1.0
        selm[r + 1, P * j + 64:P * (j + 1)] = 1.0
    selm = selm.astype(BF)

    in_maps = []
    for c in range(NCORES):
        kvt_c = np.zeros((DM, YW), np.float32)
        lo = (c - 1) * CH
        hi = (c + 2) * CH
        src_lo, src_hi = max(lo, 0), min(hi, L)
        dst_lo = src_lo - lo
        kvt_c[:, dst_lo:dst_lo + (src_hi - src_lo)] = kvT[:, src_lo:src_hi]
        in_maps.append({
            "QT": np.ascontiguousarray(qT[:, c * CH:(c + 1) * CH]),
            "KVT": kvt_c.astype(BF),
            "WQ": WQs,
            "WVK": WVK,
            "WO": WOb,
            "SEL": selm,
        })

    nc = _get_nc()
    res = bass_utils.run_bass_kernel_spmd(
        nc, in_maps, core_ids=list(range(NCORES)), trace=_trace)
    if _trace:
        _CACHE["last_result"] = res

    out = np.concatenate([r["OUT"] for r in res.results], axis=0)
    return out.reshape(B, L, DM).astype(np.float32)


# revision 16
# speedup vs baseline: 1.6407x; 1.1351x over previous
"""Local (windowed) attention with shared KV head — TRN2 Bass kernel, v3.

Problem: b=1, L=4096, d_model=1024, n_head=16, d_head=64, w=512.
  qp = (q@Wq)/8; k,v = kv@Wkv; per 512-chunk attention over {prev,self,next}
  chunks with zero-padded edges (softmax includes exp(0)=1 terms for pads);
  out = ctx @ Wo.

Sharding: sequence-parallel over the 8 chunks, one chunk per NeuronCore.
Each core recomputes the K/V projection for its 3-chunk halo (no
collectives). Edge cores receive zero-filled halo slices, which reproduces
the reference's zero-padding exactly.

v3 (from the 322us fp32r baseline, via the 222us v2):
  - all matmuls bf16 (fp32r streams ~2 cyc/row on HW; bf16 1 cyc/row)
  - exp writes bf16 probs directly from ScalarE; Exp is the only table
    function used -> exactly one ACT_TABLE_LOAD
  - inputs are pre-tiled host-side into [128, n*cols] layouts so the whole
    preamble is 8 large DMAs (DMA issue on the sync engine costs ~590ns
    per descriptor, which made 41 small DMAs the preamble bottleneck)
  - softmax 1/Z on the vector engine in a transposed [128, 4*n_head] f32
    layout (DVE reciprocal cost scales with the free dim: 0.5us for all
    16 heads vs 3.4us for a single [1,512] row); Z rows travel through
    two tiny partition-remap DMAs
  - zinv broadcast via K=16 selection-matrix matmuls; normalization of
    pairs 0..5 is interleaved into pair 7's score groups, pairs 6..7 at
    the tail; the PE never idles long enough for HAM to re-throttle
  - q-projection f-outer in the preamble (pairs 0,1) and interleaved into
    the attention phase (pairs 2..7); output projection i-outer at tail
"""

import numpy as np

B, L, DM, NH, DH, W = 1, 4096, 1024, 16, 64, 512
NCORES = 8
CH = L // NCORES        # 512 tokens per core
YW = 3 * W              # 1536 halo positions
P = 128
NF = DM // P            # 8 feature tiles
NY = YW // P            # 12 y tiles
NPAIR = NH // 2         # 8 head pairs
NGRP = NY // 2          # 6 score groups of 2 y-tiles

_CACHE = {}


def _build():
    import concourse.mybir as mybir
    import concourse.tile as tile
    from concourse import bacc
    from concourse.masks import make_identity
    from contextlib import ExitStack

    F32 = mybir.dt.float32
    BF16 = mybir.dt.bfloat16
    EXP = mybir.ActivationFunctionType.Exp

    nc = bacc.Bacc("TRN2", target_bir_lowering=False, debug=False)
    # host-pre-tiled layouts, all partition-major [128, tiles*cols]:
    #   QT  [p, f*512 + x]  = q^T[128f+p, x]        (pre-scaled by 1/8 via WQ)
    #   KVT [p, f*1536 + y] = kv^T halo[128f+p, y]
    #   WQ  [p, m*1024 + f*128 + j] = (Wq/8)[128f+p, 128m+j]   (m-major!)
    #   WVK [p, f*128 + j]  = [Wv|Wk][128f+p, j]
    #   WO  [p, i*1024 + c] = Wo[128i+p, c]
    QT = nc.dram_tensor("QT", [P, NF * CH], BF16, kind="ExternalInput")
    KVT = nc.dram_tensor("KVT", [P, NF * YW], BF16, kind="ExternalInput")
    WQ = nc.dram_tensor("WQ", [P, DM * NF], BF16, kind="ExternalInput")
    WVK = nc.dram_tensor("WVK", [P, NF * P], BF16, kind="ExternalInput")
    WO = nc.dram_tensor("WO", [P, DM * NF], BF16, kind="ExternalInput")
    SEL = nc.dram_tensor("SEL", [16, P * NPAIR], BF16, kind="ExternalInput")
    OUT = nc.dram_tensor("OUT", [CH, DM], F32, kind="ExternalOutput")

    with tile.TileContext(nc) as tc, ExitStack() as ctx:
        perm = ctx.enter_context(tc.tile_pool(name="perm", bufs=1))

        identb = perm.tile([64, 64], BF16, tag="identb")
        make_identity(nc, identb[:])

        # --- persistent SBUF tiles
        wvk = perm.tile([P, NF * P], BF16, tag="wvk")
        # wq split into (pairs 0,1) + (pairs 2..7) so the first qp matmuls
        # only gate on the smaller leading DMA
        wqA = perm.tile([P, DM * 2], BF16, tag="wqA")
        wqB = perm.tile([P, DM * 6], BF16, tag="wqB")
        wo = perm.tile([P, DM * NF], BF16, tag="wo")
        qt = perm.tile([P, NF * CH], BF16, tag="qt")
        sel = perm.tile([16, P * NPAIR], BF16, tag="sel")
        k3T2 = perm.tile([P, YW], BF16, tag="k3T2")
        vTs = perm.tile([64, YW], BF16, tag="vTs")
        v65 = [perm.tile([P, 65], BF16, tag=f"v65_{t}", name=f"v65_{t}") for t in range(NY)]
        qpT = [perm.tile([P, CH], BF16, tag=f"qpT{m}", name=f"qpT{m}") for m in range(NF)]
        # pre-normalization ctx (f32, rows 0:64) + denominator (row 64)
        stq = [perm.tile([65, W], F32, tag=f"stq{h}", name=f"stq{h}") for h in range(NH)]
        # Z in transposed layout: head h at cols 4h:4h+4 (x = 4*p + c)
        zpackT = perm.tile([P, 4 * NH], F32, tag="zpackT")
        zinvT = perm.tile([P, 4 * NH], BF16, tag="zinvT")
        zinv = perm.tile([16, W], BF16, tag="zinv")
        # normalized ctx pair tiles (lhsT of the output projection)
        pairctx = [perm.tile([P, W], BF16, tag=f"pctx{i}", name=f"pctx{i}")
                   for i in range(NPAIR)]

        # ---------- preamble: 8 big DMAs + kv/q projections, f-outer --------
        with ExitStack() as pre:
            kvtp = pre.enter_context(tc.tile_pool(name="kvt", bufs=1))
            kvps = pre.enter_context(tc.tile_pool(name="kvps", bufs=1, space="PSUM"))
            qps01 = pre.enter_context(tc.tile_pool(name="qps01", bufs=1, space="PSUM"))
            tpp = pre.enter_context(tc.tile_pool(name="tpps", bufs=2, space="PSUM"))

            HK = NF * YW // 2
            kvtA = kvtp.tile([P, HK], BF16, tag="kvtA")
            kvtB = kvtp.tile([P, HK], BF16, tag="kvtB")
            nc.sync.dma_start(wvk[:], WVK.ap()[:, :])
            nc.sync.dma_start(kvtA[:], KVT.ap()[:, 0:HK])
            nc.sync.dma_start(qt[:], QT.ap()[:, :])
            nc.sync.dma_start(wqA[:], WQ.ap()[:, 0:2 * DM])
            nc.sync.dma_start(kvtB[:], KVT.ap()[:, HK:2 * HK])
            nc.sync.dma_start(wqB[:], WQ.ap()[:, 2 * DM:NF * DM])
            nc.sync.dma_start(sel[:], SEL.ap()[:, :])
            nc.sync.dma_start(wo[:], WO.ap()[:, :])
            nc.vector.memset(zinv[:], 1.0)

            def kvs(f, n):
                t = kvtA if f < 4 else kvtB
                base = YW * (f % 4)
                return t[:, base + W * n:base + W * (n + 1)]

            def wqs(m, f):
                t = wqA if m < 2 else wqB
                base = DM * (m if m < 2 else m - 2)
                return t[:, base + P * f:base + P * (f + 1)]

            ps_n = [kvps.tile([P, W], F32, tag=f"n{n}", name=f"kvpn{n}")
                    for n in range(3)]
            qp01 = [qps01.tile([P, CH], F32, tag=f"q{m}", name=f"qp01_{m}")
                    for m in range(2)]
            for f in range(NF):
                st, sp = (f == 0), (f == NF - 1)
                for n in range(3):
                    nc.tensor.matmul(ps_n[n][:], wvk[:, P * f:P * (f + 1)],
                                     kvs(f, n),
                                     start=st, stop=sp, skip_group_check=True)
                for m in range(2):
                    nc.tensor.matmul(qp01[m][:], wqs(m, f),
                                     qt[:, CH * f:CH * (f + 1)], start=st,
                                     stop=sp, skip_group_check=True)

            for n in range(3):
                ns = slice(W * n, W * (n + 1))
                nc.vector.tensor_copy(vTs[:, ns], ps_n[n][0:64, :])
                nc.vector.tensor_copy(k3T2[64:128, ns], ps_n[n][64:128, :])
            for m in range(2):
                nc.vector.tensor_copy(qpT[m][:], qp01[m][:])
            # duplicate kT into the low partition half (partition remap DMA)
            nc.sync.dma_start(k3T2[0:64, :], k3T2[64:128, :])
            # v65 tiles: PE transpose of vT + ones column
            for t in range(NY):
                tp = tpp.tile([P, 64], BF16, tag="tp")
                nc.tensor.transpose(tp[:], vTs[:, P * t:P * (t + 1)],
                                    identb[:])
                nc.vector.tensor_copy(v65[t][:, 0:64], tp[:])
                nc.vector.memset(v65[t][:, 64:65], 1.0)

        # ---------- attention ------------------------------------------------
        with ExitStack() as att:
            scp = att.enter_context(tc.tile_pool(name="scps", bufs=2, space="PSUM"))
            cxp = att.enter_context(tc.tile_pool(name="cxps", bufs=2, space="PSUM"))
            ptp = att.enter_context(tc.tile_pool(name="pt", bufs=4))
            tbp = att.enter_context(tc.tile_pool(name="tbp", bufs=2))
            qpx = ExitStack()
            qpp = qpx.enter_context(tc.tile_pool(name="qpps", bufs=1, space="PSUM"))
            zbcp = None

            def norm_pair(j):
                # zinv rows 2j/2j+1 broadcast to 64 partitions via selection-
                # matrix matmuls (K=16), then ctxn = ctx * zinv
                zbA = zbcp.tile([64, W], F32, tag="zb")
                zbB = zbcp.tile([64, W], F32, tag="zb")
                nc.tensor.matmul(zbA[:], sel[:, P * j:P * j + 64], zinv[:],
                                 start=True, stop=True, skip_group_check=True)
                nc.tensor.matmul(zbB[:], sel[:, P * j + 64:P * (j + 1)],
                                 zinv[:],
                                 start=True, stop=True, skip_group_check=True)
                with nc.allow_low_precision(reason="ctx feeds bf16 matmul"):
                    nc.vector.tensor_mul(pairctx[j][0:64, :], stq[2 * j][0:64, :],
                                         zbA[:])
                    tmpB = tbp.tile([64, W], BF16, tag="tb")
                    nc.vector.tensor_mul(tmpB[:], stq[2 * j + 1][0:64, :], zbB[:])
                nc.sync.dma_start(pairctx[j][64:128, :], tmpB[:])

            for i in range(NPAIR):
                if i == NPAIR - 2:
                    # 1/Z for heads 0:12 (pairs 0..5): one DVE reciprocal in
                    # the transposed layout, then scatter to [16,512] rows
                    with nc.allow_low_precision(reason="zinv feeds bf16 matmul"):
                        nc.vector.reciprocal(zinvT[:, 0:48], zpackT[:, 0:48])
                    for h in range(12):
                        nc.sync.dma_start(zinv[h:h + 1, :],
                                          zinvT[:, 4 * h:4 * (h + 1)])
                    zbcp = att.enter_context(
                        tc.tile_pool(name="zbcp", bufs=2, space="PSUM"))
                cxA = cxp.tile([65, W], F32, tag="cx")
                cxB = cxp.tile([65, W], F32, tag="cx")
                for g in range(NGRP):
                    scA = scp.tile([P, 2 * W], F32, tag="sc")
                    scB = scp.tile([P, 2 * W], F32, tag="sc")
                    for t in range(2):
                        y = 2 * g + t
                        ys = slice(P * y, P * (y + 1))
                        ts_ = slice(W * t, W * (t + 1))
                        nc.tensor.matmul(scA[:, ts_], k3T2[0:64, ys],
                                         qpT[i][0:64, :], start=True, stop=True,
                                         tile_position=(0, 0),
                                         skip_group_check=True)
                        nc.tensor.matmul(scB[:, ts_], k3T2[64:128, ys],
                                         qpT[i][64:128, :], start=True, stop=True,
                                         tile_position=(64, 0),
                                         skip_group_check=True)
                    pA = ptp.tile([P, 2 * W], BF16, tag="pt")
                    pB = ptp.tile([P, 2 * W], BF16, tag="pt")
                    nc.scalar.activation(pA[:], scA[:], EXP)
                    nc.scalar.activation(pB[:], scB[:], EXP)
                    for t in range(2):
                        y = 2 * g + t
                        ts_ = slice(W * t, W * (t + 1))
                        st = (g == 0 and t == 0)
                        sp = (g == NGRP - 1 and t == 1)
                        nc.tensor.matmul(cxA[:], v65[y][:], pA[:, ts_],
                                         start=st, stop=sp,
                                         skip_group_check=True)
                        nc.tensor.matmul(cxB[:], v65[y][:], pB[:, ts_],
                                         start=st, stop=sp,
                                         skip_group_check=True)
                    # pair 7's groups carry the normalization of pairs 0..5
                    if i == NPAIR - 1:
                        norm_pair(g)
                # stage ctx+Z out of PSUM; gather Z rows into zpackT
                stA, stB = stq[2 * i], stq[2 * i + 1]
                nc.vector.tensor_copy(stA[:], cxA[:])
                nc.vector.tensor_copy(stB[:], cxB[:])
                nc.sync.dma_start(zpackT[:, 8 * i:8 * i + 4], stA[64:65, :])
                nc.sync.dma_start(zpackT[:, 8 * i + 4:8 * i + 8], stB[64:65, :])
                # q-projection for pair i+2, interleaved
                m = i + 2
                if 2 <= m < NF:
                    qps = qpp.tile([P, CH], F32, tag="qp")
                    for f in range(NF):
                        nc.tensor.matmul(qps[:], wqs(m, f),
                                         qt[:, CH * f:CH * (f + 1)],
                                         start=(f == 0), stop=(f == NF - 1),
                                         skip_group_check=True)
                    nc.vector.tensor_copy(qpT[m][:], qps[:])
                if i == NPAIR - 3:
                    qpx.close()   # free the qp PSUM bank for zb tiles

            # pairs 6, 7: 1/Z for heads 12:16, then normalization
            with nc.allow_low_precision(reason="zinv feeds bf16 matmul"):
                nc.vector.reciprocal(zinvT[:, 48:64], zpackT[:, 48:64])
            for h in range(12, 16):
                nc.sync.dma_start(zinv[h:h + 1, :], zinvT[:, 4 * h:4 * (h + 1)])
            norm_pair(NPAIR - 2)
            norm_pair(NPAIR - 1)

        # ---------- output projection: out[x,o] = sum_i ctxn[i].T @ wo[i] ----
        with tc.tile_pool(name="opps", bufs=1, space="PSUM") as opp, \
             tc.tile_pool(name="osb", bufs=4) as osb:
            ops = [[opp.tile([P, W], F32, tag=f"o{x}{o}", name=f"o{x}{o}")
                    for o in range(2)] for x in range(4)]
            for i in range(NPAIR):
                for x in range(4):
                    xs = slice(P * x, P * (x + 1))
                    for o in range(2):
                        nc.tensor.matmul(ops[x][o][:], pairctx[i][:, xs],
                                         wo[:, DM * i + W * o:DM * i + W * (o + 1)],
                                         start=(i == 0), stop=(i == NPAIR - 1),
                                         skip_group_check=True)
            for x in range(4):
                xs = slice(P * x, P * (x + 1))
                for o in range(2):
                    os_ = slice(W * o, W * (o + 1))
                    ot = osb.tile([P, W], F32, tag="os")
                    nc.scalar.copy(ot[:], ops[x][o][:])
                    nc.sync.dma_start(OUT.ap()[xs, os_], ot[:])

    nc.compile()
    return nc


def _get_nc():
    if "nc" not in _CACHE:
        _CACHE["nc"] = _build()
    return _CACHE["nc"]


def _tile_rows(a, cols):
    """[NF*128, cols] row-major -> [128, NF*cols] partition-major tiles."""
    return np.ascontiguousarray(
        a.reshape(NF, P, cols).transpose(1, 0, 2).reshape(P, NF * cols))


def kernel(q, kv, Wq, Wkv, Wo, w=None, _trace=False):
    import ml_dtypes
    from concourse import bass_utils

    BF = ml_dtypes.bfloat16

    q = np.asarray(q, np.float32).reshape(L, DM)
    kv = np.asarray(kv, np.float32).reshape(L, DM)
    Wq = np.asarray(Wq, np.float32)
    Wkv = np.asarray(Wkv, np.float32)
    Wo = np.asarray(Wo, np.float32)

    qT = np.ascontiguousarray(q.T)                       # [DM, L]
    kvT = np.ascontiguousarray(kv.T)                     # [DM, L]
    # WQ in m-major tiling: [p, m*1024 + f*128 + j] = (Wq/8)[128f+p, 128m+j]
    WQs = (Wq / np.sqrt(DH)).reshape(NF, P, NF, P).transpose(1, 2, 0, 3)
    WQs = np.ascontiguousarray(WQs.reshape(P, DM * NF)).astype(BF)
    WVKc = np.concatenate([Wkv[:, DH:], Wkv[:, :DH]], axis=1)   # [Wv | Wk]
    WVKt = _tile_rows(WVKc, P).astype(BF)
    WOt = _tile_rows(Wo, DM).astype(BF)

    # selection matrix: SEL[2j, 128j + c] = 1 for c < 64 (head A);
    #                   SEL[2j+1, ...] for c >= 64 (head B)
    selm = np.zeros((16, P * NPAIR), np.float32)
    for j in range(NPAIR):
        selm[2 * j, P * j:P * j + 64] = 1.0
        selm[2 * j + 1, P * j + 64:P * (j + 1)] = 1.0
    selm = selm.astype(BF)

    in_maps = []
    for c in range(NCORES):
        kvt_c = np.zeros((DM, YW), np.float32)
        lo = (c - 1) * CH
        hi = (c + 2) * CH
        src_lo, src_hi = max(lo, 0), min(hi, L)
        dst_lo = src_lo - lo
        kvt_c[:, dst_lo:dst_lo + (src_hi - src_lo)] = kvT[:, src_lo:src_hi]
        in_maps.append({
            "QT": _tile_rows(np.ascontiguousarray(qT[:, c * CH:(c + 1) * CH]),
                             CH).astype(BF),
            "KVT": _tile_rows(kvt_c, YW).astype(BF),
            "WQ": WQs,
            "WVK": WVKt,
            "WO": WOt,
            "SEL": selm,
        })

    nc = _get_nc()
    res = bass_utils.run_bass_kernel_spmd(
        nc, in_maps, core_ids=list(range(NCORES)), trace=_trace)
    if _trace:
        _CACHE["last_result"] = res

    out = np.concatenate([r["OUT"] for r in res.results], axis=0)
    return out.reshape(B, L, DM).astype(np.float32)


# revision 22
# speedup vs baseline: 1.6908x; 1.0305x over previous
"""Local (windowed) attention with shared KV head — TRN2 Bass kernel, v3.

Problem: b=1, L=4096, d_model=1024, n_head=16, d_head=64, w=512.
  qp = (q@Wq)/8; k,v = kv@Wkv; per 512-chunk attention over {prev,self,next}
  chunks with zero-padded edges (softmax includes exp(0)=1 terms for pads);
  out = ctx @ Wo.

Sharding: sequence-parallel over the 8 chunks, one chunk per NeuronCore.
Each core recomputes the K/V projection for its 3-chunk halo (no
collectives). Edge cores receive zero-filled halo slices, which reproduces
the reference's zero-padding exactly.

v3 (from the 322us fp32r baseline, via the 222us v2):
  - all matmuls bf16 (fp32r streams ~2 cyc/row on HW; bf16 1 cyc/row)
  - exp writes bf16 probs directly from ScalarE; Exp is the only table
    function used -> exactly one ACT_TABLE_LOAD
  - inputs are pre-tiled host-side into [128, n*cols] layouts so the whole
    preamble is 8 large DMAs (DMA issue on the sync engine costs ~590ns
    per descriptor, which made 41 small DMAs the preamble bottleneck)
  - softmax 1/Z on the vector engine in a transposed [128, 4*n_head] f32
    layout (DVE reciprocal cost scales with the free dim: 0.5us for all
    16 heads vs 3.4us for a single [1,512] row); Z rows travel through
    two tiny partition-remap DMAs
  - zinv broadcast via K=16 selection-matrix matmuls; normalization of
    pairs 0..5 is interleaved into pair 7's score groups, pairs 6..7 at
    the tail; the PE never idles long enough for HAM to re-throttle
  - q-projection f-outer in the preamble (pairs 0,1) and interleaved into
    the attention phase (pairs 2..7); output projection i-outer at tail
"""

import numpy as np

B, L, DM, NH, DH, W = 1, 4096, 1024, 16, 64, 512
NCORES = 8
CH = L // NCORES        # 512 tokens per core
YW = 3 * W              # 1536 halo positions
P = 128
NF = DM // P            # 8 feature tiles
NY = YW // P            # 12 y tiles
NPAIR = NH // 2         # 8 head pairs
NGRP = NY // 2          # 6 score groups of 2 y-tiles

_CACHE = {}


def _build():
    import concourse.mybir as mybir
    import concourse.tile as tile
    from concourse import bacc
    from concourse.masks import make_identity
    from contextlib import ExitStack

    F32 = mybir.dt.float32
    BF16 = mybir.dt.bfloat16
    EXP = mybir.ActivationFunctionType.Exp

    nc = bacc.Bacc("TRN2", target_bir_lowering=False, debug=False)
    # host-pre-tiled layouts, all partition-major [128, tiles*cols]:
    #   QT  [p, f*512 + x]  = q^T[128f+p, x]        (pre-scaled by 1/8 via WQ)
    #   KVT [p, f*1536 + y] = kv^T halo[128f+p, y]
    #   WQ  [p, m*1024 + f*128 + j] = (Wq/8)[128f+p, 128m+j]   (m-major!)
    #   WVK [p, f*128 + j]  = [Wv|Wk][128f+p, j]
    #   WO  [p, i*1024 + c] = Wo[128i+p, c]
    QT = nc.dram_tensor("QT", [P, NF * CH], BF16, kind="ExternalInput")
    KVT = nc.dram_tensor("KVT", [P, NF * YW], BF16, kind="ExternalInput")
    WQ = nc.dram_tensor("WQ", [P, DM * NF], BF16, kind="ExternalInput")
    WVK = nc.dram_tensor("WVK", [P, NF * P], BF16, kind="ExternalInput")
    WO = nc.dram_tensor("WO", [P, DM * NF], BF16, kind="ExternalInput")
    SEL = nc.dram_tensor("SEL", [16, P * NPAIR], BF16, kind="ExternalInput")
    OUT = nc.dram_tensor("OUT", [CH, DM], F32, kind="ExternalOutput")

    with tile.TileContext(nc) as tc, ExitStack() as ctx:
        perm = ctx.enter_context(tc.tile_pool(name="perm", bufs=1))

        identb = perm.tile([64, 64], BF16, tag="identb")
        make_identity(nc, identb[:])

        # --- persistent SBUF tiles
        wvk = perm.tile([P, NF * P], BF16, tag="wvk")
        # wq split into (pairs 0,1) + (pairs 2..7) so the first qp matmuls
        # only gate on the smaller leading DMA
        wqA = perm.tile([P, DM * 2], BF16, tag="wqA")
        wqB = perm.tile([P, DM * 6], BF16, tag="wqB")
        wo = perm.tile([P, DM * NF], BF16, tag="wo")
        qt = perm.tile([P, NF * CH], BF16, tag="qt")
        sel = perm.tile([16, P * NPAIR], BF16, tag="sel")
        k3T2 = perm.tile([P, YW], BF16, tag="k3T2")
        vTs = perm.tile([64, YW], BF16, tag="vTs")
        v65 = [perm.tile([P, 65], BF16, tag=f"v65_{t}", name=f"v65_{t}") for t in range(NY)]
        qpT = [perm.tile([P, CH], BF16, tag=f"qpT{m}", name=f"qpT{m}") for m in range(NF)]
        # pre-normalization ctx (f32, rows 0:64) + denominator (row 64)
        stq = [perm.tile([65, W], F32, tag=f"stq{h}", name=f"stq{h}") for h in range(NH)]
        # Z in transposed layout: head h at cols 4h:4h+4 (x = 4*p + c)
        zpackT = perm.tile([P, 4 * NH], F32, tag="zpackT")
        zinvT = perm.tile([P, 4 * NH], BF16, tag="zinvT")
        zinv = perm.tile([16, W], BF16, tag="zinv")
        # normalized ctx pair tiles (lhsT of the output projection)
        pairctx = [perm.tile([P, W], BF16, tag=f"pctx{i}", name=f"pctx{i}")
                   for i in range(NPAIR)]

        # ---------- preamble: 8 big DMAs + kv/q projections, f-outer --------
        with ExitStack() as pre:
            kvtp = pre.enter_context(tc.tile_pool(name="kvt", bufs=1))
            kvps = pre.enter_context(tc.tile_pool(name="kvps", bufs=1, space="PSUM"))
            qps01 = pre.enter_context(tc.tile_pool(name="qps01", bufs=1, space="PSUM"))
            tpp = pre.enter_context(tc.tile_pool(name="tpps", bufs=2, space="PSUM"))

            HK = NF * YW // 2
            kvtA = kvtp.tile([P, HK], BF16, tag="kvtA")
            kvtB = kvtp.tile([P, HK], BF16, tag="kvtB")
            nc.sync.dma_start(wvk[:], WVK.ap()[:, :])
            nc.sync.dma_start(kvtA[:], KVT.ap()[:, 0:HK])
            nc.sync.dma_start(kvtB[:], KVT.ap()[:, HK:2 * HK])
            nc.sync.dma_start(qt[:], QT.ap()[:, :])
            nc.sync.dma_start(wqA[:], WQ.ap()[:, 0:2 * DM])
            nc.sync.dma_start(wqB[:], WQ.ap()[:, 2 * DM:NF * DM])
            nc.sync.dma_start(sel[:], SEL.ap()[:, :])
            nc.sync.dma_start(wo[:], WO.ap()[:, :])
            nc.vector.memset(zinv[:], 1.0)

            def kvs(f, n):
                t = kvtA if f < 4 else kvtB
                base = YW * (f % 4)
                return t[:, base + W * n:base + W * (n + 1)]

            def wqs(m, f):
                t = wqA if m < 2 else wqB
                base = DM * (m if m < 2 else m - 2)
                return t[:, base + P * f:base + P * (f + 1)]

            ps_n = [kvps.tile([P, W], F32, tag=f"n{n}", name=f"kvpn{n}")
                    for n in range(3)]
            qp01 = [qps01.tile([P, CH], F32, tag=f"q{m}", name=f"qp01_{m}")
                    for m in range(2)]
            for f in range(NF):
                st, sp = (f == 0), (f == NF - 1)
                for n in range(3):
                    nc.tensor.matmul(ps_n[n][:], wvk[:, P * f:P * (f + 1)],
                                     kvs(f, n),
                                     start=st, stop=sp, skip_group_check=True)
            for m in range(2):
                for f in range(NF):
                    nc.tensor.matmul(qp01[m][:], wqs(m, f),
                                     qt[:, CH * f:CH * (f + 1)],
                                     start=(f == 0), stop=(f == NF - 1),
                                     skip_group_check=True)

            for n in range(3):
                ns = slice(W * n, W * (n + 1))
                nc.vector.tensor_copy(vTs[:, ns], ps_n[n][0:64, :])
                nc.vector.tensor_copy(k3T2[64:128, ns], ps_n[n][64:128, :])
            for m in range(2):
                nc.vector.tensor_copy(qpT[m][:], qp01[m][:])
            # duplicate kT into the low partition half (partition remap DMA)
            nc.sync.dma_start(k3T2[0:64, :], k3T2[64:128, :])
            # v65 tiles: PE transpose of vT + ones column
            for t in range(NY):
                tp = tpp.tile([P, 64], BF16, tag="tp")
                nc.tensor.transpose(tp[:], vTs[:, P * t:P * (t + 1)],
                                    identb[:])
                nc.vector.tensor_copy(v65[t][:, 0:64], tp[:])
                nc.vector.memset(v65[t][:, 64:65], 1.0)

        # ---------- attention ------------------------------------------------
        with ExitStack() as att2:
            tbp = att2.enter_context(tc.tile_pool(name="tbp", bufs=2))

            def norm_pair(j, zpool):
                # zinv rows 2j/2j+1 broadcast to 64 partitions via selection-
                # matrix matmuls (K=16), then ctxn = ctx * zinv
                zbA = zpool.tile([P, W], F32, tag="aux")
                zbB = zpool.tile([P, W], F32, tag="aux")
                nc.tensor.matmul(zbA[0:64, :], sel[:, P * j:P * j + 64],
                                 zinv[:],
                                 start=True, stop=True, skip_group_check=True)
                nc.tensor.matmul(zbB[0:64, :], sel[:, P * j + 64:P * (j + 1)],
                                 zinv[:],
                                 start=True, stop=True, skip_group_check=True)
                with nc.allow_low_precision(reason="ctx feeds bf16 matmul"):
                    nc.vector.tensor_mul(pairctx[j][0:64, :], stq[2 * j][0:64, :],
                                         zbA[0:64, :])
                    tmpB = tbp.tile([64, W], BF16, tag="tb")
                    nc.vector.tensor_mul(tmpB[:], stq[2 * j + 1][0:64, :],
                                         zbB[0:64, :])
                nc.sync.dma_start(pairctx[j][64:128, :], tmpB[:])

            def recip_scatter(c0, c1):
                # 1/Z in the transposed layout, then scatter to zinv rows
                with nc.allow_low_precision(reason="zinv feeds bf16 matmul"):
                    nc.vector.reciprocal(zinvT[:, 4 * c0:4 * c1],
                                         zpackT[:, 4 * c0:4 * c1])
                for h in range(c0, c1):
                    nc.sync.dma_start(zinv[h:h + 1, :],
                                      zinvT[:, 4 * h:4 * (h + 1)])

            with ExitStack() as att1:
                scp = att1.enter_context(tc.tile_pool(name="scps", bufs=2, space="PSUM"))
                cxp = att1.enter_context(tc.tile_pool(name="cxps", bufs=2, space="PSUM"))
                ptp = att1.enter_context(tc.tile_pool(name="pt", bufs=4))
                # shared [128,512] PSUM pool: qp accumulators (pairs 2..7)
                # and zb broadcast tiles (pairs 0..5) — 2 banks total
                auxp = att1.enter_context(tc.tile_pool(name="auxp", bufs=2, space="PSUM"))

                for i in range(NPAIR):
                    if i == NPAIR - 2:
                        recip_scatter(0, 12)   # pairs 0..5
                    cxA = cxp.tile([65, W], F32, tag="cx")
                    cxB = cxp.tile([65, W], F32, tag="cx")
                    for g in range(NGRP):
                        scA = scp.tile([P, 2 * W], F32, tag="sc")
                        scB = scp.tile([P, 2 * W], F32, tag="sc")
                        for t in range(2):
                            y = 2 * g + t
                            ys = slice(P * y, P * (y + 1))
                            ts_ = slice(W * t, W * (t + 1))
                            nc.tensor.matmul(scA[:, ts_], k3T2[0:64, ys],
                                             qpT[i][0:64, :], start=True,
                                             stop=True, tile_position=(0, 0),
                                             skip_group_check=True)
                            nc.tensor.matmul(scB[:, ts_], k3T2[64:128, ys],
                                             qpT[i][64:128, :], start=True,
                                             stop=True, tile_position=(64, 0),
                                             skip_group_check=True)
                        pA = ptp.tile([P, 2 * W], BF16, tag="pt")
                        pB = ptp.tile([P, 2 * W], BF16, tag="pt")
                        nc.scalar.activation(pA[:], scA[:], EXP)
                        nc.scalar.activation(pB[:], scB[:], EXP)
                        for t in range(2):
                            y = 2 * g + t
                            ts_ = slice(W * t, W * (t + 1))
                            st = (g == 0 and t == 0)
                            sp = (g == NGRP - 1 and t == 1)
                            nc.tensor.matmul(cxA[:], v65[y][:], pA[:, ts_],
                                             start=st, stop=sp,
                                             skip_group_check=True)
                            nc.tensor.matmul(cxB[:], v65[y][:], pB[:, ts_],
                                             start=st, stop=sp,
                                             skip_group_check=True)
                        # pair 7's groups carry the normalization of pairs 0..5
                        if i == NPAIR - 1:
                            norm_pair(g, auxp)
                    # stage ctx+Z out of PSUM; gather Z rows into zpackT
                    stA, stB = stq[2 * i], stq[2 * i + 1]
                    nc.vector.tensor_copy(stA[:], cxA[:])
                    nc.vector.tensor_copy(stB[:], cxB[:])
                    nc.sync.dma_start(zpackT[:, 8 * i:8 * i + 4], stA[64:65, :])
                    nc.sync.dma_start(zpackT[:, 8 * i + 4:8 * i + 8],
                                      stB[64:65, :])
                    if i >= NPAIR - 2:
                        # pair 6/7's own 1/Z right after its Z rows land
                        recip_scatter(2 * i, 2 * i + 2)
                    # q-projection for pair i+2, interleaved
                    m = i + 2
                    if 2 <= m < NF:
                        qps = auxp.tile([P, W], F32, tag="aux")
                        for f in range(NF):
                            nc.tensor.matmul(qps[:], wqs(m, f),
                                             qt[:, CH * f:CH * (f + 1)],
                                             start=(f == 0), stop=(f == NF - 1),
                                             skip_group_check=True)
                        nc.vector.tensor_copy(qpT[m][:], qps[:])

            # att1 closed: scores/ctx PSUM banks are free from here
            # ---------- output projection: out[x,o] = sum_i ctxn[i].T@wo[i] --
            # x-o outer, i inner, 2-bank pipelined PSUM; pairs 6/7's
            # normalization is slotted behind the first tile's i<7 matmuls
            with tc.tile_pool(name="zbc2", bufs=2, space="PSUM") as zbc2, \
                 tc.tile_pool(name="opps", bufs=2, space="PSUM") as opp, \
                 tc.tile_pool(name="osb", bufs=3) as osb:
                norm_pair(NPAIR - 2, zbc2)
                first = True
                for x in range(4):
                    xs = slice(P * x, P * (x + 1))
                    for o in range(2):
                        os_ = slice(W * o, W * (o + 1))
                        ps = opp.tile([P, W], F32, tag="op")
                        for i in range(NPAIR - 1):
                            nc.tensor.matmul(ps[:], pairctx[i][:, xs],
                                             wo[:, DM * i + W * o:DM * i + W * (o + 1)],
                                             start=(i == 0), stop=False,
                                             skip_group_check=True)
                        if first:
                            norm_pair(NPAIR - 1, zbc2)
                            first = False
                        i = NPAIR - 1
                        nc.tensor.matmul(ps[:], pairctx[i][:, xs],
                                         wo[:, DM * i + W * o:DM * i + W * (o + 1)],
                                         start=False, stop=True,
                                         skip_group_check=True)
                        ot = osb.tile([P, W], F32, tag="os")
                        nc.vector.tensor_copy(ot[:], ps[:])
                        nc.sync.dma_start(OUT.ap()[xs, os_], ot[:])

    nc.compile()
    return nc


def _get_nc():
    if "nc" not in _CACHE:
        _CACHE["nc"] = _build()
    return _CACHE["nc"]


def _tile_rows(a, cols):
    """[NF*128, cols] row-major -> [128, NF*cols] partition-major tiles."""
    return np.ascontiguousarray(
        a.reshape(NF, P, cols).transpose(1, 0, 2).reshape(P, NF * cols))


def kernel(q, kv, Wq, Wkv, Wo, w=None, _trace=False):
    import ml_dtypes
    from concourse import bass_utils

    BF = ml_dtypes.bfloat16

    q = np.asarray(q, np.float32).reshape(L, DM)
    kv = np.asarray(kv, np.float32).reshape(L, DM)
    Wq = np.asarray(Wq, np.float32)
    Wkv = np.asarray(Wkv, np.float32)
    Wo = np.asarray(Wo, np.float32)

    qT = np.ascontiguousarray(q.T)                       # [DM, L]
    kvT = np.ascontiguousarray(kv.T)                     # [DM, L]
    # WQ in m-major tiling: [p, m*1024 + f*128 + j] = (Wq/8)[128f+p, 128m+j]
    WQs = (Wq / np.sqrt(DH)).reshape(NF, P, NF, P).transpose(1, 2, 0, 3)
    WQs = np.ascontiguousarray(WQs.reshape(P, DM * NF)).astype(BF)
    WVKc = np.concatenate([Wkv[:, DH:], Wkv[:, :DH]], axis=1)   # [Wv | Wk]
    WVKt = _tile_rows(WVKc, P).astype(BF)
    WOt = _tile_rows(Wo, DM).astype(BF)

    # selection matrix: SEL[2j, 128j + c] = 1 for c < 64 (head A);
    #                   SEL[2j+1, ...] for c >= 64 (head B)
    selm = np.zeros((16, P * NPAIR), np.float32)
    for j in range(NPAIR):
        selm[2 * j, P * j:P * j + 64] = 1.0
        selm[2 * j + 1, P * j + 64:P * (j + 1)] = 1.0
    selm = selm.astype(BF)

    in_maps = []
    for c in range(NCORES):
        kvt_c = np.zeros((DM, YW), np.float32)
        lo = (c - 1) * CH
        hi = (c + 2) * CH
        src_lo, src_hi = max(lo, 0), min(hi, L)
        dst_lo = src_lo - lo
        kvt_c[:, dst_lo:dst_lo + (src_hi - src_lo)] = kvT[:, src_lo:src_hi]
        in_maps.append({
            "QT": _tile_rows(np.ascontiguousarray(qT[:, c * CH:(c + 1) * CH]),
                             CH).astype(BF),
            "KVT": _tile_rows(kvt_c, YW).astype(BF),
            "WQ": WQs,
            "WVK": WVKt,
            "WO": WOt,
            "SEL": selm,
        })

    nc = _get_nc()
    res = bass_utils.run_bass_kernel_spmd(
        nc, in_maps, core_ids=list(range(NCORES)), trace=_trace)
    if _trace:
        _CACHE["last_result"] = res

    out = np.concatenate([r["OUT"] for r in res.results], axis=0)
    return out.reshape(B, L, DM).astype(np.float32)
